# revision 29
# baseline (speedup 1.0000x reference)
"""Trainium2 kernel for AutoPatchOverLapModel3D (3D patch overlap-add / fold).

Math: out[b,p,y0,y1,y2] = (1/CM[y0,y1,y2]) * sum_{j0,j1,j2}
        x[b, y0-j0, y1-j1, (y2-j2)%64, p, j0, j1, j2]
i.e. a stride-1 overlap-add of 5x5x5 patches; axes 0/1 zero-padded,
axis 2 circular; CM is the separable patch-count normalizer.

Strategy (8 NeuronCores, SPMD):
  - The patch index n factors as n = col*64 + i2 with col=(b*10+i0)*28+i1
    (560 columns of 64 circularly-coupled patches each). Shard 70
    columns per core; groups of 128 patches (2 columns) per tile.
  - The circular j2 fold is a +/-2 partition rotation. Rather than
    matmul against shift matrices, the HOST pre-rotates each of the 5
    j2 tap blocks (a free numpy permutation while casting to fp16), so
    on device every tap is partition-aligned and the fold is a plain
    5-way elementwise sum, split across engines to stay under the DMA
    roofline: 3 blocks summed on TensorE via identity-weight matmuls
    accumulating in PSUM (full-rate fp16, 1 cycle/row), 2 blocks added
    directly by the DVE in its 2x fp16 mode.
  - The j1 fold lands in a per-frame fp16 accumulator (one DVE add per
    group into a y1f window; u=1 columns stored shifted by -1 and fixed
    up on the host), fp16 halving both DVE cost and output traffic.
  - The tiny j0 overlap-add across planes and the CM division run on
    the host. Input quantization fp32->fp16 costs ~3e-4 rel error,
    far inside the 2e-2 gate.
"""

import numpy as np

B, X0, X1, X2, P = 2, 10, 28, 64, 20
PK = 5  # patch edge
Y0, Y1, Y2 = 14, 32, 64
NCOL = B * X0 * X1            # 560 (b,i0,i1) columns
NCORES = 8
COLS_PER_CORE = NCOL // NCORES  # 70
PATCH_VEC = P * PK * PK * PK    # 2500
FREE = P * PK * PK              # 500 = (j1, p, j0)
NGROUPS = NCOL * X2 // 128      # 280 groups of 128 patches (2 cols)
GROUPS = COLS_PER_CORE * X2 // 128  # 35 groups per core
FRAMES = 5                      # half-plane frames per core (14 cols each)
GROUPS_PER_FRAME = 7
YF = 17                         # y1f span stored per frame: 2q+j1, q<7

_CACHE = {}


def _prepare_inputs(x):
    """Cast to fp16 and pre-rotate the five j2 tap blocks.

    Returns per-core input dicts. Block k of 500 (j1,p,j0) floats holds
    tap j2=k rotated so SBUF partition m=(u,y2) already contains
    x[col(u), i2=(y2-k+2)%64, :, :, :, k]; the on-device j2 fold is
    then a plain 5-way aligned sum (j1-major blocks keep the DVE's
    innermost stride at 1 for its 2x fp16 mode).
    """
    xf = np.ascontiguousarray(x, np.float32).astype(np.float16)
    A = xf.reshape(NCOL * X2, P, PK, PK, PK)
    A = A.transpose(0, 4, 3, 1, 2)                # n, j2, j1, p, j0
    A = A.reshape(NGROUPS, 2, 64, PK, FREE)       # g, u, i2, j2, (j1 p j0)
    Bt = np.empty((NGROUPS, 2, 64, PATCH_VEC), np.float16)
    for j2 in range(PK):
        Bt[:, :, :, j2 * FREE:(j2 + 1) * FREE] = np.roll(
            A[:, :, :, j2, :], j2 - 2, axis=2
        )
    Bt = Bt.reshape(NGROUPS, 128, PATCH_VEC)
    # partition-major flat layout: a core's 35 groups share contiguous
    # per-partition lines, so group-pair DMAs move 10 KB/partition and
    # pairs may span frame boundaries.
    Bt = Bt.reshape(NCORES, GROUPS, 128, PATCH_VEC)
    Bt = np.ascontiguousarray(Bt.transpose(0, 2, 1, 3)).reshape(
        NCORES, 128, GROUPS * PATCH_VEC
    )
    wnp = np.eye(128, dtype=np.float16)
    return [{"xs": Bt[c], "w": wnp} for c in range(NCORES)]


def _kernel_body(tc, xs, w, out):
    import concourse.mybir as mybir

    nc = tc.nc
    f16 = xs.dtype
    f32 = mybir.dt.float32
    with (
        tc.tile_pool(name="wpool", bufs=1) as wpool,
        tc.tile_pool(name="xpool", bufs=10) as xpool,
        tc.tile_pool(name="spool", bufs=8) as spool,
        tc.tile_pool(name="accpool", bufs=3) as accpool,
        tc.tile_pool(name="pspool", bufs=6, space="PSUM") as pspool,
    ):
        wt = wpool.tile([128, 128], f16)
        nc.sync.dma_start(out=wt[:, :], in_=w[:, :])
        # 5 half-plane frames of 7 groups (14 columns) each; frame
        # boundaries are half-plane aligned on every core (70 % 14 == 0),
        # keeping the program SPMD-uniform. Group pairs are loaded as
        # one 10 KB/partition DMA and may span frame boundaries.
        accs = {}
        avs = {}
        for r in range(GROUPS // 2 + 1):
            n = 2 if r < GROUPS // 2 else 1
            xt = xpool.tile([128, n * PATCH_VEC], f16)
            nc.sync.dma_start(
                out=xt[:, :],
                in_=xs[:, 2 * r * PATCH_VEC:(2 * r + n) * PATCH_VEC],
            )
            for m in range(n):
                g = 2 * r + m
                h, q = divmod(g, GROUPS_PER_FRAME)
                if q == 0:
                    acc = accpool.tile([128, 100 * YF], f16)
                    nc.gpsimd.memset(acc[:, :], 0.0)
                    accs[h] = acc
                    avs[h] = acc[:, :].rearrange("a (y f) -> a y f", y=YF)
                # writeback of the previous frame, delayed two groups so
                # its semaphore wait is already satisfied when the
                # Scalar HWDGE sequencer reaches it (no FIFO blocking).
                if q == 2 and h > 0:
                    nc.scalar.dma_start(
                        out=out[h - 1, :, :], in_=accs[h - 1][:, :]
                    )
                base = m * PATCH_VEC
                # j2 fold: identity-weight matmuls accumulate pe_taps
                # pre-rotated blocks in PSUM at full fp16 rate; ScalarE
                # (closest to PSUM) evacuates to fp16 SBUF so the DVE
                # never reads PSUM. The terminal group shifts one tap
                # from PE to DVE: its MM+ACT leg shortens while the
                # longer DVE presum chain still hides under it, cutting
                # the final writeback's critical path.
                pe_taps = 2 if g == GROUPS - 1 else 3
                ps = pspool.tile([128, FREE], f32)
                for k in range(pe_taps):
                    nc.tensor.matmul(
                        ps[:, :],
                        wt[:, :],
                        xt[:, base + k * FREE:base + (k + 1) * FREE],
                        start=(k == 0),
                        stop=(k == pe_taps - 1),
                    )
                s1 = spool.tile([128, FREE], f16)
                nc.scalar.copy(s1[:, :], ps[:, :])
                # remaining taps presummed in fp16 2x mode (independent
                # of the matmul chain, so it overlaps the PE)
                s2 = spool.tile([128, FREE], f16)
                nc.vector.tensor_add(
                    s2[:, :],
                    xt[:, base + pe_taps * FREE:base + (pe_taps + 1) * FREE],
                    xt[:, base + (pe_taps + 1) * FREE:base + (pe_taps + 2) * FREE],
                )
                for k in range(pe_taps + 2, 5):
                    nc.vector.tensor_add(
                        s2[:, :], s2[:, :],
                        xt[:, base + k * FREE:base + (k + 1) * FREE],
                    )
                # j1 fold: column i1 = 2q+u lands at stored row y1f =
                # 2q+j1 for BOTH u-blocks (u=1 stored shifted by -1,
                # fixed up on the host).
                lo = 2 * q
                dst = avs[h][:, lo:lo + 5, :]
                nc.vector.tensor_add(
                    dst, dst, s2[:, :].rearrange("a (j f) -> a j f", j=5)
                )
                nc.vector.tensor_add(
                    dst, dst, s1[:, :].rearrange("a (j f) -> a j f", j=5)
                )
                # tail overlap: rows 0-11 of the last frame are final
                # once group q=5's adds land, so most of its writeback
                # overlaps the final group's compute chain.
                if h == FRAMES - 1 and q == 5:
                    nc.scalar.dma_start(
                        out=out[h, :, 0:1200], in_=accs[h][:, 0:1200]
                    )
        nc.scalar.dma_start(
            out=out[FRAMES - 1, :, 1200:], in_=accs[FRAMES - 1][:, 1200:]
        )


def _build_nc():
    import concourse.bacc as bacc
    import concourse.mybir as mybir
    import concourse.tile as tile

    nc = bacc.Bacc(
        "TRN2",
        target_bir_lowering=False,
        debug=False,
        enable_asserts=True,
        num_devices=NCORES,
    )
    f16 = mybir.dt.float16
    xs = nc.declare_dram_parameter(
        "xs", [128, GROUPS * PATCH_VEC], f16, isOutput=False
    )
    w = nc.declare_dram_parameter("w", [128, 128], f16, isOutput=False)
    out = nc.declare_dram_parameter("out", [FRAMES, 128, 100 * YF], f16, isOutput=True)

    with tile.TileContext(nc) as tc:
        _kernel_body(tc, xs, w, out)
    nc.compile()
    return nc


def _counting_matrix():
    c0 = np.zeros(Y0, np.float32)
    for i0 in range(X0):
        c0[i0:i0 + PK] += 1
    c1 = np.zeros(Y1, np.float32)
    for i1 in range(X1):
        c1[i1:i1 + PK] += 1
    return c0[:, None, None] * c1[None, :, None] * 5.0


def _stitch(oc):
    # oc[c, h] holds half-plane H=5c+h partials [(u, y2), (y1f, p, j0)];
    # place at y1 = 14*(H%2) + u + y1f, y0 = i0 + j0.
    ocr = oc.reshape(NCORES * FRAMES, 2, 64, YF, P, PK).astype(np.float32)
    ocr = ocr.transpose(0, 1, 4, 5, 3, 2)                   # H,u,p,j0,y1f,y2
    out = np.zeros((B, P, Y0, Y1, Y2), np.float32)
    for H in range(NCORES * FRAMES):
        gp, half = divmod(H, 2)
        b, i0 = divmod(gp, X0)
        y1lo = (X1 // 2) * half
        for u in range(2):
            out[b, :, i0:i0 + PK, y1lo + u:y1lo + u + YF, :] += ocr[H, u]
    out /= _counting_matrix()
    return out


def kernel(x: np.ndarray) -> np.ndarray:
    from concourse.bass_utils import run_bass_kernel_spmd

    if "nc" not in _CACHE:
        _CACHE["nc"] = _build_nc()
    nc = _CACHE["nc"]

    in_maps = _prepare_inputs(x)
    res = run_bass_kernel_spmd(nc, in_maps, list(range(NCORES)))
    oc = np.stack([res.results[c]["out"] for c in range(NCORES)], axis=0)
    return _stitch(oc)
